# revision 31
# baseline (speedup 1.0000x reference)
"""AttnBlock (GroupNorm + single-head self-attention + residual) on 8 TRN2 cores.

Sharding: core = 2*b + half. Each core handles one batch element (b = core//2)
and one half of the query rows (half = core%2). The half is implemented by
swapping the token halves of x[b] host-side, so every core runs the identical
SPMD program computing outputs for local tokens [0, 2048).

Per-core device program (C=256 channels, N=4096 tokens, NH=2048 query rows):
  - GroupNorm(32 groups) via bn_stats + small PE matmuls for the cross-
    partition group reduction; 1/std via a DVE rsqrt bit-hack + one Newton
    step so the ACT engine never loads the sqrt table (its only function set
    is exp, loaded once at t=0).
  - k/q projections (bf16 matmuls) are converted straight out of PSUM into
    fp8e4m3 with bias added (DVE), packed [128, 2, N] (channel c on partition
    c%128, plane c//128) so S^T runs in fp8 DoubleRow (0.5 PE cycles/row).
    vT tiles [128, 2, 257] fp8 (even/odd token planes, col 256 memset to 1 so
    PV also produces the softmax denominator). DVE conversion order
    q0 k0 q1 k1 q2 k2 q3 k3 v0 k4 v1 k5 v2 k6 v3 k7 v4..v15 keeps the exp
    stream fed from ~14us while vT lands before the first PV needs it.
  - S^T fp8 DoubleRow into 2-bank PSUM tiles; softmax exponentials run
    1024-wide on BOTH fp engines concurrently: most on ACT
    (exp(S/16-2) -> e4m3), a tuned subset on DVE as a Schraudolph fast-exp
    (bits = S*a+b rounded to uint8, bitcast e5m2 - safe two-sided range, and
    the PE accepts mixed e5m2/e4m3 DoubleRow operands). The e^-2 scaling
    cancels in the softmax ratio; fast-exp's ~5% per-weight error is noise
    far below the output tolerance (measured end-to-end ~1e-6).
  - PV in fp8 DoubleRow accumulated per 128-query group; the software
    pipeline displaces each chunk's PV into the next chunk's exp window
    (half 0 in gaps 0-7, half 1 in gaps 8-15, two PSUM banks rotating), with
    the last chunk's first half squeezed into its own window so the tail
    after the final exp is only its second PV half + projection.
  - Finish: divide by denom, PE-transpose o^T -> o (pair-packed, one DVE
    copy per 256 columns), out = (wo@o + bo') + x fused in one DVE op.
    bv is folded host-side into bo' = wo@bv + bo (exact).

Engine balance (cost model): ACT ~54us of exps, DVE ~57us + 12 fast-exps,
PE ~50us. Accumulation always fp32 in PSUM; residual path fp32.
"""

import math

import ml_dtypes
import numpy as np

import concourse.bass as bass
import concourse.tile as tile
from concourse import bacc, mybir
from concourse.bass import ts, ds
from concourse.bass_utils import run_bass_kernel_spmd

B, C, W = 4, 256, 64
N = W * W            # 4096 tokens
NH = N // 2          # 2048 query rows per core
GROUPS = 32
GSIZE = C // GROUPS  # 8 channels per group
EPS = 1e-6
P = 128
CT = C // P          # 2 channel tiles
NCH = 512            # n-chunk width for S^T / projections
SCALE = 1.0 / 16.0   # 1/sqrt(C)

F32 = mybir.dt.float32
BF = mybir.dt.bfloat16
F8 = mybir.dt.float8e4
F8E5 = mybir.dt.float8e5
U8 = mybir.dt.uint8
I32 = mybir.dt.int32
PMT = 16  # packed key-token tiles (256 tokens each, even/odd planes)

AF = mybir.ActivationFunctionType
ALU = mybir.AluOpType

# Schraudolph fast-exp into e5m2 bits: bits = round(S*FE_MUL + FE_OFF).
# value(bits) ~= exp(S/16 - 2); range-safe for |S/16| < 8 on both sides.
FE_MUL = SCALE * 4.0 / math.log(2.0)
FE_OFF = 60.0 - 2.0 * 4.0 / math.log(2.0) - 0.172

RSQRT_MAGIC = 0x5F3759DF

# which key tiles' exps run on DVE (fast-exp), per query chunk; chosen on
# gaps where the DVE has no finish/projection work queued
DVE_J = {0: (), 1: (6, 8, 10, 12), 2: (6, 8, 10, 12), 3: (6, 8, 10, 12)}

_CACHE = {}


def _build_program():
    nc = bacc.Bacc("TRN2", target_bir_lowering=False, debug=False, num_devices=8)

    xb = nc.dram_tensor("xb", [C, NH], F32, kind="ExternalInput").ap()
    xlb = nc.dram_tensor("xlb", [C, NH], BF, kind="ExternalInput").ap()
    xhb = nc.dram_tensor("xhb", [C, NH], BF, kind="ExternalInput").ap()
    wqT = nc.dram_tensor("wqT", [C, C], BF, kind="ExternalInput").ap()
    wkT = nc.dram_tensor("wkT", [C, C], BF, kind="ExternalInput").ap()
    wvT = nc.dram_tensor("wvT", [C, C], BF, kind="ExternalInput").ap()
    woT = nc.dram_tensor("woT", [C, C], BF, kind="ExternalInput").ap()
    # all small fp32 constants packed in one tensor: one DMA instead of ~15.
    # layout: [0:10] per-ct (bq, bk, bo', gamma, beta), [10:26] mfwd,
    # [26:154] mbwd (partitions 0:16 valid)
    CPK = 10 + 16 + P
    cpack = nc.dram_tensor("cpack", [P, CPK], F32, kind="ExternalInput").ap()
    ident = nc.dram_tensor("ident", [P, P], BF, kind="ExternalInput").ap()
    out = nc.dram_tensor("out", [C, NH], F32, kind="ExternalOutput").ap()

    GT = GROUPS // CT  # 16 groups per channel tile

    with tile.TileContext(nc) as tc:
        with (
            tc.tile_pool(name="persist", bufs=1) as persist,
            tc.tile_pool(name="consts", bufs=1) as consts,
            tc.tile_pool(name="vt_pool", bufs=PMT) as vt_pool,
        ):
            # ---- x load first: GroupNorm heads the dependency chain.
            # Queues: sync=xl ct0, gpsimd=xl/xh ct1, scalar=xh ct0+consts.
            x_sb = [persist.tile([P, NH], F32, tag=f"x{ct}", name=f"x{ct}") for ct in range(CT)]
            xl_sb = [persist.tile([P, NH], BF, tag=f"xl{ct}", name=f"xl{ct}") for ct in range(CT)]
            xh_sb = [persist.tile([P, NH], BF, tag=f"xh{ct}", name=f"xh{ct}") for ct in range(CT)]
            for hh in range(2):
                nc.sync.dma_start(
                    out=xl_sb[0][:, ts(hh, NH // 2)], in_=xlb[ts(0, P), ts(hh, NH // 2)]
                )
                nc.gpsimd.dma_start(
                    out=xl_sb[1][:, ts(hh, NH // 2)], in_=xlb[ts(1, P), ts(hh, NH // 2)]
                )
                nc.scalar.dma_start(
                    out=xh_sb[0][:, ts(hh, NH // 2)], in_=xhb[ts(0, P), ts(hh, NH // 2)]
                )
                nc.gpsimd.dma_start(
                    out=xh_sb[1][:, ts(hh, NH // 2)], in_=xhb[ts(1, P), ts(hh, NH // 2)]
                )
            cpack_sb = consts.tile([P, CPK], F32)
            nc.scalar.dma_start(out=cpack_sb, in_=cpack)

            # ---- constants (sync queue, behind xl ct0) --------------------
            wq_sb = consts.tile([P, CT, C], BF)
            wk_sb = consts.tile([P, CT, C], BF)
            wv_sb = consts.tile([P, CT, C], BF)
            wo_sb = consts.tile([P, CT, C], BF)
            for ct in range(CT):
                nc.sync.dma_start(out=wk_sb[:, ct, :], in_=wkT[ts(ct, P), :])
                nc.sync.dma_start(out=wq_sb[:, ct, :], in_=wqT[ts(ct, P), :])
            for ct in range(CT):
                nc.sync.dma_start(out=wv_sb[:, ct, :], in_=wvT[ts(ct, P), :])
                nc.sync.dma_start(out=wo_sb[:, ct, :], in_=woT[ts(ct, P), :])
            ident_sb = consts.tile([P, P], BF)
            nc.scalar.dma_start(out=ident_sb, in_=ident)
            for hh in range(2):
                for ct in range(CT):
                    eng = nc.sync if ct == 0 else nc.gpsimd
                    eng.dma_start(
                        out=x_sb[ct][:, ts(hh, NH // 2)],
                        in_=xb[ts(ct, P), ts(hh, NH // 2)],
                    )
            # constant bias inside exp keeps fp8 attention weights in range
            # (max score/16 ~ 5.5 -> exp up to ~450 overflows e4m3); the e^-2
            # factor cancels exactly in the softmax ratio.
            nexp_sb = consts.tile([P, 1], F32)
            nc.vector.memset(nexp_sb, -2.0)
            # views into the packed constants
            bq_sb = cpack_sb[:, 0:CT]
            bk_sb = cpack_sb[:, CT : 2 * CT]
            bo_sb = cpack_sb[:, 2 * CT : 3 * CT]
            gam_sb = cpack_sb[:, 3 * CT : 4 * CT]
            bet_sb = cpack_sb[:, 4 * CT : 5 * CT]
            mfwd_sb = cpack_sb[:, 10 : 10 + GT]
            mbwd_sb = cpack_sb[0:GT, 26 : 26 + P]

            # ---- persistent activations -----------------------------------
            # q8/k8: channel c lives at (partition c % 128, plane c // 128)
            # so fp8 DoubleRow matmuls contract all 256 channels at once.
            q8_sb = persist.tile([P, CT, NH], F8, tag="q8", name="q8")
            k8_sb = persist.tile([P, CT, N], F8, tag="k8", name="k8")
            h_sb = [persist.tile([P, N], BF, tag=f"h{ct}", name=f"h{ct}") for ct in range(CT)]
            oT_sb = [persist.tile([P, NH], BF, tag=f"oT{ct}", name=f"oT{ct}") for ct in range(CT)]
            vt_tiles = [vt_pool.tile([P, 2, C + 1], F8, tag="vt", name=f"vt{j}") for j in range(PMT)]
            # denominator column: constant 1 plane, written once (gpsimd)
            for j in range(PMT):
                nc.gpsimd.memset(vt_tiles[j][:, :, C : C + 1], 1.0)

            # ---- GroupNorm -------------------------------------------------
            with (
                tc.tile_pool(name="gn_pool", bufs=3) as gn_pool,
                tc.tile_pool(name="gn_psum", bufs=1, space="PSUM") as gn_psum,
            ):
                ab_consts = []
                st2s = []
                for ct in range(CT):
                    xr = xl_sb[ct].rearrange("p (s f) -> p s f", f=512)
                    xhr = xh_sb[ct].rearrange("p (s f) -> p s f", f=512)
                    st6 = gn_pool.tile([P, N // 512, 6], F32, tag=f"st6{ct}", name=f"st6{ct}")
                    for s in range(NH // 512):
                        nc.vector.bn_stats(out=st6[:, s, :], in_=xr[:, s, :])
                    for s in range(NH // 512):
                        nc.vector.bn_stats(
                            out=st6[:, NH // 512 + s, :], in_=xhr[:, s, :]
                        )
                    mv = gn_pool.tile([P, 2], F32, tag=f"mv{ct}", name=f"mv{ct}")
                    nc.vector.bn_aggr(out=mv, in_=st6)
                    # st2 = (mean_c, E[x^2]_c)
                    st2 = gn_pool.tile([P, 2], F32, tag=f"st2{ct}", name=f"st2{ct}")
                    nc.vector.tensor_copy(out=st2[:, 0:1], in_=mv[:, 0:1])
                    msq = gn_pool.tile([P, 1], F32, tag=f"msq{ct}", name=f"msq{ct}")
                    nc.vector.tensor_mul(out=msq, in0=mv[:, 0:1], in1=mv[:, 0:1])
                    nc.vector.tensor_add(out=st2[:, 1:2], in0=mv[:, 1:2], in1=msq)
                    st2s.append(st2)
                for ct in range(CT):
                    st2 = st2s[ct]
                    # per-group (mu, E[x^2]) via 1/8-weighted column sums
                    psum_g = gn_psum.tile([GT, 2], F32, tag="pg")
                    nc.tensor.matmul(psum_g, lhsT=mfwd_sb, rhs=st2, start=True, stop=True)
                    gs = gn_pool.tile([GT, 2], F32, tag="gs")
                    nc.vector.tensor_copy(out=gs[:, 0:1], in_=psum_g[:, 0:1])
                    mq = gn_pool.tile([GT, 1], F32, tag="mq")
                    nc.vector.tensor_mul(out=mq, in0=gs[:, 0:1], in1=gs[:, 0:1])
                    # v = var + eps = (E[x^2] + eps) - mu^2
                    gv = gn_pool.tile([GT, 1], F32, tag="gv")
                    nc.vector.scalar_tensor_tensor(
                        out=gv, in0=psum_g[:, 1:2], scalar=EPS, in1=mq,
                        op0=ALU.add, op1=ALU.subtract,
                    )
                    # 1/sqrt(v): bit-hack seed + one Newton step (all DVE,
                    # keeps the ACT table pinned on exp). Final rel err
                    # ~2e-5; the chain is 6 tiny ops off the exp path.
                    iv = gn_pool.tile([GT, 1], I32, tag="iv")
                    nc.vector.tensor_scalar(
                        out=iv, in0=gv.bitcast(I32), scalar1=1,
                        scalar2=None, op0=ALU.arith_shift_right,
                    )
                    y0i = gn_pool.tile([GT, 1], I32, tag="y0i")
                    nc.vector.tensor_scalar(
                        out=y0i, in0=iv, scalar1=-1, scalar2=RSQRT_MAGIC,
                        op0=ALU.mult, op1=ALU.add,
                    )
                    y0 = y0i.bitcast(F32)
                    t1 = gn_pool.tile([GT, 1], F32, tag="t1")
                    nc.vector.tensor_mul(out=t1, in0=y0, in1=y0)
                    nc.vector.tensor_mul(out=t1, in0=t1, in1=gv)
                    nc.vector.tensor_scalar(
                        out=t1, in0=t1, scalar1=-0.5, scalar2=1.5,
                        op0=ALU.mult, op1=ALU.add,
                    )
                    nc.vector.tensor_mul(out=gs[:, 1:2], in0=y0, in1=t1)
                    # broadcast group stats back to channels
                    psum_bc = gn_psum.tile([P, 2], F32, tag="pbc")
                    nc.tensor.matmul(psum_bc, lhsT=mbwd_sb, rhs=gs, start=True, stop=True)
                    amul = consts.tile([P, 1], F32, tag=f"amul{ct}", name=f"amul{ct}")
                    badd = consts.tile([P, 1], F32, tag=f"badd{ct}", name=f"badd{ct}")
                    nc.vector.tensor_mul(out=amul, in0=psum_bc[:, 1:2], in1=gam_sb[:, ct : ct + 1])
                    nc.vector.tensor_mul(out=badd, in0=psum_bc[:, 0:1], in1=amul)
                    nc.vector.tensor_sub(out=badd, in0=bet_sb[:, ct : ct + 1], in1=badd)
                    ab_consts.append((amul, badd))
                # h = x*A + B (bf16 in/out SBUF -> 4x DVE mode); only the
                # first token piece is written here, the rest interleaves
                # with the projection loop so q0/k0 conversions start sooner
                def h_piece(s4):
                    for ct in range(CT):
                        amul, badd = ab_consts[ct]
                        src_t = xl_sb[ct] if s4 < 2 else xh_sb[ct]
                        sl = ts(s4 % 2, NH // 2)
                        nc.vector.tensor_scalar(
                            out=h_sb[ct][:, ts(s4, N // 4)],
                            in0=src_t[:, sl],
                            scalar1=amul,
                            scalar2=badd,
                            op0=ALU.mult,
                            op1=ALU.add,
                        )

                h_piece(0)

            # ---- projections + attention, software-pipelined --------------
            NCHUNKS = NH // NCH
            with (
                tc.tile_pool(name="p_pool", bufs=64) as p_pool,
                tc.tile_pool(name="s_psum", bufs=2, space="PSUM") as s_psum,
                tc.tile_pool(name="o_psum", bufs=2, space="PSUM") as o_psum,
                tc.tile_pool(name="o_pool", bufs=6) as o_pool,
                tc.tile_pool(name="r_pool", bufs=8) as r_pool,
                tc.tile_pool(name="out_pool", bufs=4) as out_pool,
            ):
                mm_psum = tc.alloc_tile_pool(name="mm_psum", bufs=2, space="PSUM")

                def proj_qk(w_sb, b_sb, dst, ch, on_act=False):
                    for mo in range(CT):
                        psk = mm_psum.tile([P, NCH], F32, tag="psk", name="psk")
                        for ct in range(CT):
                            nc.tensor.matmul(
                                psk,
                                lhsT=w_sb[:, ct, ts(mo, P)],
                                rhs=h_sb[ct][:, ts(ch, NCH)],
                                start=(ct == 0),
                                stop=(ct == CT - 1),
                            )
                        if on_act:
                            # the idle ACT head converts the first key chunk
                            # so the first exp isn't waiting on the DVE queue
                            nc.scalar.activation(
                                out=dst[:, mo, ts(ch, NCH)],
                                in_=psk,
                                func=AF.Identity,
                                bias=b_sb[:, mo : mo + 1],
                                scale=1.0,
                            )
                        else:
                            nc.vector.tensor_scalar_add(
                                out=dst[:, mo, ts(ch, NCH)],
                                in0=psk,
                                scalar1=b_sb[:, mo : mo + 1],
                            )

                def proj_v(j):
                    psv = mm_psum.tile([P, 2, C], F32, tag="psk", name="psv")
                    for parity in range(2):
                        for ct in range(CT):
                            hsl = h_sb[ct][:, ds(j * 2 * P, 2 * P)].rearrange(
                                "p (m two) -> p two m", two=2
                            )
                            nc.tensor.matmul(
                                psv[:, parity, :],
                                lhsT=hsl[:, parity, :],
                                rhs=wv_sb[:, ct, :],
                                start=(ct == 0),
                                stop=(ct == CT - 1),
                            )
                    nc.vector.tensor_copy(out=vt_tiles[j][:, :, 0:C], in_=psv)

                pts_all = [[None] * PMT for _ in range(NCHUNKS)]
                dve_pend = []

                def _ksl(j):
                    return k8_sb[:, :, ds(j * 2 * P, 2 * P)].rearrange(
                        "p r (m two) -> p r two m", two=2
                    )

                def _dve_parity(ch, j, pt, parity, dpool):
                    # fast-exp path: its S matmul gets a private PSUM bank so
                    # the ACT exp stream's two-slot rotation never skips
                    psd = dpool.tile([P, NCH], F32, tag="dves", name=f"psd{ch}_{j}_{parity}")
                    nc.tensor.matmul(
                        psd,
                        lhsT=_ksl(j)[:, :, parity, :],
                        rhs=q8_sb[:, :, ts(ch, NCH)],
                        start=True,
                        stop=True,
                        perf_mode=mybir.MatmulPerfMode.DoubleRow,
                    )
                    nc.vector.tensor_scalar(
                        out=pt.bitcast(U8)[:, parity, :],
                        in0=psd,
                        scalar1=FE_MUL,
                        scalar2=FE_OFF,
                        op0=ALU.mult,
                        op1=ALU.add,
                    )

                def dve_drain():
                    while dve_pend:
                        ch, j, pt, dpool = dve_pend.pop(0)
                        _dve_parity(ch, j, pt, 1, dpool)

                def s_exp(ch, j, dpool=None):
                    on_dve = dpool is not None and j in DVE_J[ch]
                    pt = p_pool.tile(
                        [P, 2, NCH], F8E5 if on_dve else F8,
                        tag="pt", name=f"pt{ch}_{j}",
                    )
                    pts_all[ch][j] = pt
                    if on_dve:
                        # parity 0 now; parity 1 next gap (the single bank
                        # round-trips through the DVE in between)
                        _dve_parity(ch, j, pt, 0, dpool)
                        dve_pend.append((ch, j, pt, dpool))
                        return
                    pss = s_psum.tile([P, 2, NCH], F32, tag="pss")
                    for parity in range(2):
                        nc.tensor.matmul(
                            pss[:, parity, :],
                            lhsT=_ksl(j)[:, :, parity, :],
                            rhs=q8_sb[:, :, ts(ch, NCH)],
                            start=True,
                            stop=True,
                            perf_mode=mybir.MatmulPerfMode.DoubleRow,
                        )
                    nc.scalar.activation(
                        out=pt, in_=pss, func=AF.Exp, scale=SCALE, bias=nexp_sb
                    )

                # PV bookkeeping: psos[(ch, half)] -> [pso_g0, pso_g1]
                psos = {}

                def pv2(ch, half, jj):
                    key = (ch, half)
                    if key not in psos:
                        psos[key] = [
                            o_psum.tile([P, C + 1], F32, tag="pso", name=f"pso{ch}_{half}_{g}")
                            for g in range(2)
                        ]
                    for g in range(2):
                        nt = 2 * half + g
                        nc.tensor.matmul(
                            psos[key][g],
                            lhsT=pts_all[ch][jj][:, :, ts(nt, P)],
                            rhs=vt_tiles[jj],
                            start=(jj == 0),
                            stop=(jj == PMT - 1),
                            perf_mode=mybir.MatmulPerfMode.DoubleRow,
                        )

                osb_pend = {}

                def finish_dve(ch, half):
                    # divide the accumulated o^T by the softmax denominator
                    pp = psos.pop((ch, half))
                    osbs = []
                    for g in range(2):
                        nt = 2 * half + g
                        rec = r_pool.tile([P, 1], F32, tag="rec", name=f"rec{ch}_{nt}")
                        nc.vector.reciprocal(out=rec, in_=pp[g][:, C : C + 1])
                        osb = o_pool.tile([P, C], BF, tag="osb", name=f"osb{ch}_{nt}")
                        nc.vector.tensor_scalar_mul(out=osb, in0=pp[g][:, 0:C], scalar1=rec)
                        osbs.append(osb)
                    osb_pend[(ch, half)] = osbs

                def finish_pe(ch, half, pool, pst_tag="psf"):
                    # transpose o^T -> o (emitted a gap later than the DVE
                    # part so the PE never blocks waiting on osb)
                    osbs = osb_pend.pop((ch, half))
                    for cc in range(CT):
                        pst = pool.tile([P, 2, P], BF, tag=pst_tag, name=f"pst{ch}{half}{cc}")
                        for g in range(2):
                            nc.tensor.transpose(pst[:, g, :], osbs[g][:, ts(cc, P)], ident_sb)
                        nc.vector.tensor_copy(
                            out=oT_sb[cc][:, ds(ch * NCH + half * 2 * P, 2 * P)],
                            in_=pst,
                        )

                def proj_mo(ch, mo, pool, tag="psf"):
                    psf = pool.tile([P, NCH], F32, tag=tag, name=f"psj{ch}{mo}")
                    for ct in range(CT):
                        nc.tensor.matmul(
                            psf,
                            lhsT=wo_sb[:, ct, ts(mo, P)],
                            rhs=oT_sb[ct][:, ts(ch, NCH)],
                            start=(ct == 0),
                            stop=(ct == CT - 1),
                        )
                    fs = out_pool.tile([P, NCH], F32, tag="fs", name=f"fs{ch}{mo}")
                    nc.vector.scalar_tensor_tensor(
                        out=fs,
                        in0=psf,
                        scalar=bo_sb[:, mo : mo + 1],
                        in1=x_sb[mo][:, ts(ch, NCH)],
                        op0=ALU.add,
                        op1=ALU.add,
                    )
                    eng = nc.sync if mo == 0 else nc.gpsimd
                    eng.dma_start(out=out[ts(mo, P), ts(ch, NCH)], in_=fs)

                # -- window 0: projections + chunk-0 exps interleaved.
                # DVE conversion order: q0, k0(ACT), k1..k7 (h pieces
                # slotted in), then q1..q3 and v interleaved so the key
                # tiles keep pace with the exps and vT lands just ahead of
                # chunk 0's PV in window 1.
                proj_qk(wq_sb, bq_sb, q8_sb, 0)
                for pc in range(8):
                    if pc >= 4:
                        proj_v(pc - 4)
                    proj_qk(wk_sb, bk_sb, k8_sb, pc, on_act=(pc == 0))
                    s_exp(0, 2 * pc)
                    s_exp(0, 2 * pc + 1)
                    if pc in (1, 3, 5):
                        h_piece(pc // 2 + 1)
                proj_qk(wq_sb, bq_sb, q8_sb, 1)
                proj_v(4)
                proj_v(5)
                proj_qk(wq_sb, bq_sb, q8_sb, 2)
                proj_v(6)
                proj_v(7)
                proj_qk(wq_sb, bq_sb, q8_sb, 3)
                for j in range(8, PMT):
                    proj_v(j)
                mm_psum.release()

                # -- windows 1..3: exps + displaced PV pipeline -------------
                # Gap map for window ch (prev = ch-1, pp = ch-2):
                #   g0:    pv(prev,h0) 0-1   + finish_dve(pp, h1)
                #   g1:    pv(prev,h0) 2-3   + finish_pe(pp, h1)
                #   g2:    pv(prev,h0) 4-5   + proj_mo(pp, 0)
                #   g3:    pv(prev,h0) 6-7   + proj_mo(pp, 1)
                #   g4-7:  pv(prev,h0) 8-15
                #   g8:    finish_dve(prev, h0)
                #   g9:    finish_pe(prev, h0) + pv(prev,h1) 0-1
                #   g10-15: pv(prev,h1) 2-15 (2-3 per gap); in the last
                #          window also chunk-3 h0 accumulations (exp-gated)
                # (tf_psum and d_psum reuse the two banks mm_psum released)
                with (
                    tc.tile_pool(name="tf_psum", bufs=1, space="PSUM") as tf_psum,
                    tc.tile_pool(name="d_psum", bufs=1, space="PSUM") as d_psum,
                ):
                    lc = NCHUNKS - 1
                    h0_pend = list(range(PMT))  # last chunk's half-0 PV queue
                    spill = None  # (ch, mo) projection emitted at next gap 0
                    for ch in range(1, NCHUNKS):
                        prev = ch - 1
                        last = ch == lc
                        for j in range(PMT):
                            s_exp(ch, j, d_psum)
                            if j == 0 and spill is not None:
                                proj_mo(spill[0], spill[1], d_psum, tag="dves")
                                spill = None
                            if j < 4:
                                for jj in range(4 * j, 4 * j + 4):
                                    pv2(prev, 0, jj)
                            elif j == 4:
                                finish_dve(prev, 0)
                            elif j == 5:
                                finish_pe(prev, 0, tf_psum)
                                pv2(prev, 1, 0)
                                pv2(prev, 1, 1)
                            elif j <= 12:
                                pv2(prev, 1, 2 * j - 10)
                                pv2(prev, 1, 2 * j - 9)
                            elif j == 13:
                                finish_dve(prev, 1)
                            elif j == 14:
                                finish_pe(prev, 1, tf_psum)
                            else:
                                proj_mo(prev, 0, tf_psum)
                                spill = (prev, 1)
                            if last and j >= 6:
                                # chunk-3 half-0: accumulate exp-ready tiles
                                # (fast-exp tiles are complete a gap later)
                                take = [
                                    jj for jj in h0_pend
                                    if jj < j and (jj not in DVE_J[lc] or jj < j - 1)
                                ][:3]
                                for jj in take:
                                    h0_pend.remove(jj)
                                    pv2(lc, 0, jj)
                            dve_drain()

                    # tail: the exp rotation is over, so chunk-3's half-1 PV
                    # accumulates in the two dead s_psum slots (separate
                    # tiles - slice-sharing one tile serializes the
                    # accumulation on the PSUM read-modify-write latency).
                    # Only its last tiles gate on the final exps; nothing
                    # waits on chunk-2's banks.
                    proj_mo(spill[0], spill[1], d_psum, tag="dves")
                    psos[(lc, 1)] = [
                        s_psum.tile([P, 2, NCH], F32, tag="pss", name=f"pso31_{g}")[
                            :, 0, 0 : C + 1
                        ]
                        for g in range(2)
                    ]
                    for jj in range(PMT):
                        pv2(lc, 1, jj)
                    for jj in h0_pend:
                        pv2(lc, 0, jj)
                    finish_dve(lc, 0)
                    finish_dve(lc, 1)
                    finish_pe(lc, 0, tf_psum)
                    finish_pe(lc, 1, o_psum, pst_tag="pso")
                    proj_mo(lc, 0, tf_psum)
                    proj_mo(lc, 1, d_psum, tag="dves")

    nc.compile()
    return nc


def get_program():
    if "nc" not in _CACHE:
        _CACHE["nc"] = _build_program()
    return _CACHE["nc"]


def _cpack(bq, bk, bo2, gam, bet):
    cp = np.zeros((P, 10 + 16 + P), np.float32)
    for j, v in enumerate([bq, bk, bo2, gam, bet]):
        cp[:, 2 * j : 2 * j + 2] = v.reshape(CT, P).T
    mfwd = (
        np.arange(P)[:, None] // GSIZE == np.arange(GROUPS // CT)[None, :]
    ).astype(np.float32) / GSIZE
    mbwd = (
        np.arange(GROUPS // CT)[:, None] == np.arange(P)[None, :] // GSIZE
    ).astype(np.float32)
    cp[:, 10:26] = mfwd
    cp[: GROUPS // CT, 26 : 26 + P] = mbwd
    return cp


def _make_in_maps(x, gn_gamma, gn_beta, wq, bq, wk, bk, wv, bv, wo, bo):
    f = lambda a: np.ascontiguousarray(np.asarray(a, dtype=np.float32))
    x = f(x).reshape(B, C, N)
    # v's bias is constant across tokens, so it rides through attention
    # unchanged: softmax(S) @ (v + bv) = softmax(S) @ v + bv. Fold it into
    # the output projection bias: bo' = wo @ bv + bo (exact fp32 host math).
    bo2 = f(wo) @ f(bv) + f(bo)
    shared = {
        "wqT": f(wq).T.astype(ml_dtypes.bfloat16),
        "wkT": f(wk).T.astype(ml_dtypes.bfloat16),
        "wvT": np.ascontiguousarray(f(wv).T).astype(ml_dtypes.bfloat16),
        "woT": f(wo).T.astype(ml_dtypes.bfloat16),
        "cpack": _cpack(f(bq), f(bk), bo2, f(gn_gamma), f(gn_beta)),
        "ident": np.eye(P).astype(ml_dtypes.bfloat16),
    }
    in_maps = []
    for core in range(8):
        b, half = core // 2, core % 2
        xbv = x[b]
        if half == 1:
            xbv = np.concatenate([xbv[:, NH:], xbv[:, :NH]], axis=1)
        in_maps.append(
            {
                "xb": np.ascontiguousarray(xbv[:, :NH]),
                "xlb": xbv[:, :NH].astype(ml_dtypes.bfloat16),
                "xhb": xbv[:, NH:].astype(ml_dtypes.bfloat16),
                **shared,
            }
        )
    return in_maps


def kernel(**inputs):
    nc = get_program()
    in_maps = _make_in_maps(**inputs)
    res = run_bass_kernel_spmd(nc, in_maps, list(range(8)))
    out = np.empty((B, C, N), dtype=np.float32)
    for core in range(8):
        b, half = core // 2, core % 2
        out[b, :, half * NH : (half + 1) * NH] = res.results[core]["out"]
    return out.reshape(B, C, W, W)


# revision 59
# speedup vs baseline: 1.0822x; 1.0822x over previous
"""AttnBlock (GroupNorm + single-head self-attention + residual) on 8 TRN2 cores.

Sharding: core = 2*b + half. Each core handles one batch element (b = core//2)
and one half of the query rows (half = core%2). The half is implemented by
swapping the token halves of x[b] host-side, so every core runs the identical
SPMD program computing outputs for local tokens [0, 2048).

Per-core device program (C=256 channels, N=4096 tokens, NH=2048 query rows):
  - GroupNorm(32 groups) via bn_stats + small PE matmuls for the cross-
    partition group reduction; 1/std via a DVE rsqrt bit-hack + one Newton
    step so the ACT engine never loads the sqrt table (its only function set
    is exp, loaded once at t=0).
  - GroupNorm statistics are split across engines: DVE bn_stats covers the
    first token half while the otherwise-idle ACT head accumulates sum and
    sum-of-squares of the second half (Identity/Square + accum_out, same
    table set as exp), combined into per-channel (mean, E[x^2]).
  - k/q projections (bf16 matmuls) are converted straight out of PSUM into
    fp8e4m3 with bias added (DVE; the first key chunk on the idle ACT head),
    packed [128, 2, N] (channel c on partition c%128, plane c//128) so S^T
    runs in fp8 DoubleRow (0.5 PE cycles/row). vT tiles [128, 2, 257] fp8
    (even/odd token planes, col 256 memset to 1 so PV also produces the
    softmax denominator). The conversion order q0 k0..k7 (h pieces and v
    tiles slotted between) keeps the exp stream fed from ~17us while vT
    lands before the first PV needs it.
  - S^T fp8 DoubleRow into 2-bank PSUM tiles; softmax exponentials run
    1024-wide on BOTH fp engines concurrently: most on ACT
    (exp(S/16-2) -> e4m3), a tuned subset on DVE as a Schraudolph fast-exp
    (bits = S*a+b rounded to uint8, bitcast e5m2 - safe two-sided range, and
    the PE accepts mixed e5m2/e4m3 DoubleRow operands). The e^-2 scaling
    cancels in the softmax ratio; fast-exp's ~5% per-weight error is noise
    far below the output tolerance (measured end-to-end ~1e-6).
  - PV in fp8 DoubleRow accumulated per 128-query group; the software
    pipeline displaces each chunk's PV into the next chunk's exp window
    (half 0 in gaps 0-3, half 1 in gaps 5-12, finishes and the fused output
    projection in the remaining gaps, two PSUM banks rotating). The last
    chunk's half-0 PV squeezes into its own window and its half-1 PV
    accumulates in the dead exp-rotation banks, so the tail after the final
    exp is only a finish + projection chain.
  - Finish: divide by denom, PE-transpose o^T -> o (pair-packed, one DVE
    copy per 256 columns), out = (wo@o + bo') + x fused in one DVE op.
    bv is folded host-side into bo' = wo@bv + bo (exact).

Engine balance (cost model, 89.5us total): ACT ~67us (56 exps + head
stats accumulators + first key conversion + tail divides), DVE ~60us
(conversions, finishes, 8 fast-exps), PE ~50us. Chunk 0 takes no finish
work in window 1 (the DVE is still draining conversions there); its
half-1 PV borrows the then-idle tf/d PSUM banks and all its finishes
open window 2. The last chunk's output projection runs half-granular so
the final DMAs launch per finish half. Accumulation always fp32 in PSUM;
residual path fp32. Measured rel err vs the fp32 reference: ~7e-7.
"""

import math

import ml_dtypes
import numpy as np

import concourse.bass as bass
import concourse.tile as tile
from concourse import bacc, mybir
from concourse.bass import ts, ds
from concourse.bass_utils import run_bass_kernel_spmd

B, C, W = 4, 256, 64
N = W * W            # 4096 tokens
NH = N // 2          # 2048 query rows per core
GROUPS = 32
GSIZE = C // GROUPS  # 8 channels per group
EPS = 1e-6
P = 128
CT = C // P          # 2 channel tiles
NCH = 512            # n-chunk width for S^T / projections
SCALE = 1.0 / 16.0   # 1/sqrt(C)

F32 = mybir.dt.float32
BF = mybir.dt.bfloat16
F8 = mybir.dt.float8e4
F8E5 = mybir.dt.float8e5
U8 = mybir.dt.uint8
I32 = mybir.dt.int32
PMT = 16  # packed key-token tiles (256 tokens each, even/odd planes)

AF = mybir.ActivationFunctionType
ALU = mybir.AluOpType

# Schraudolph fast-exp into e5m2 bits: bits = round(S*FE_MUL + FE_OFF).
# value(bits) ~= exp(S/16 - 2); range-safe for |S/16| < 8 on both sides.
FE_MUL = SCALE * 4.0 / math.log(2.0)
FE_OFF = 60.0 - 2.0 * 4.0 / math.log(2.0) - 0.172

RSQRT_MAGIC = 0x5F3759DF

# which key tiles' exps run on DVE (fast-exp), per query chunk; chosen on
# gaps where the DVE has no finish/projection work queued
DVE_J = {0: (), 1: (), 2: (3, 6, 9, 12), 3: (3, 6, 9, 12)}

_CACHE = {}


def _build_program():
    nc = bacc.Bacc("TRN2", target_bir_lowering=False, debug=False, num_devices=8)

    xb = nc.dram_tensor("xb", [C, NH], F32, kind="ExternalInput").ap()
    xlb = nc.dram_tensor("xlb", [C, NH], BF, kind="ExternalInput").ap()
    xhb = nc.dram_tensor("xhb", [C, NH], BF, kind="ExternalInput").ap()
    wqT = nc.dram_tensor("wqT", [C, C], BF, kind="ExternalInput").ap()
    wkT = nc.dram_tensor("wkT", [C, C], BF, kind="ExternalInput").ap()
    wvT = nc.dram_tensor("wvT", [C, C], BF, kind="ExternalInput").ap()
    woT = nc.dram_tensor("woT", [C, C], BF, kind="ExternalInput").ap()
    # all small fp32 constants packed in one tensor: one DMA instead of ~15.
    # layout: [0:10] per-ct (bq, bk, bo', gamma, beta), [10:26] mfwd,
    # [26:154] mbwd (partitions 0:16 valid)
    CPK = 10 + 16 + P
    cpack = nc.dram_tensor("cpack", [P, CPK], F32, kind="ExternalInput").ap()
    ident = nc.dram_tensor("ident", [P, P], BF, kind="ExternalInput").ap()
    out = nc.dram_tensor("out", [C, NH], F32, kind="ExternalOutput").ap()

    GT = GROUPS // CT  # 16 groups per channel tile

    with tile.TileContext(nc) as tc:
        with (
            tc.tile_pool(name="persist", bufs=1) as persist,
            tc.tile_pool(name="consts", bufs=1) as consts,
            tc.tile_pool(name="vt_pool", bufs=PMT) as vt_pool,
        ):
            # ---- x load first: GroupNorm heads the dependency chain.
            # Queues: sync=xl ct0, gpsimd=xl/xh ct1, scalar=xh ct0+consts.
            x_sb = [persist.tile([P, NH], F32, tag=f"x{ct}", name=f"x{ct}") for ct in range(CT)]
            xl_sb = [persist.tile([P, NH], BF, tag=f"xl{ct}", name=f"xl{ct}") for ct in range(CT)]
            xh_sb = [persist.tile([P, NH], BF, tag=f"xh{ct}", name=f"xh{ct}") for ct in range(CT)]
            for hh in range(2):
                nc.sync.dma_start(
                    out=xl_sb[0][:, ts(hh, NH // 2)], in_=xlb[ts(0, P), ts(hh, NH // 2)]
                )
                nc.gpsimd.dma_start(
                    out=xl_sb[1][:, ts(hh, NH // 2)], in_=xlb[ts(1, P), ts(hh, NH // 2)]
                )
                nc.scalar.dma_start(
                    out=xh_sb[0][:, ts(hh, NH // 2)], in_=xhb[ts(0, P), ts(hh, NH // 2)]
                )
                nc.gpsimd.dma_start(
                    out=xh_sb[1][:, ts(hh, NH // 2)], in_=xhb[ts(1, P), ts(hh, NH // 2)]
                )
            cpack_sb = consts.tile([P, CPK], F32)
            nc.scalar.dma_start(out=cpack_sb, in_=cpack)

            # ---- constants (sync queue, behind xl ct0) --------------------
            wq_sb = consts.tile([P, CT, C], BF)
            wk_sb = consts.tile([P, CT, C], BF)
            wv_sb = consts.tile([P, CT, C], BF)
            wo_sb = consts.tile([P, CT, C], BF)
            for ct in range(CT):
                nc.sync.dma_start(out=wk_sb[:, ct, :], in_=wkT[ts(ct, P), :])
                nc.sync.dma_start(out=wq_sb[:, ct, :], in_=wqT[ts(ct, P), :])
            for ct in range(CT):
                nc.sync.dma_start(out=wv_sb[:, ct, :], in_=wvT[ts(ct, P), :])
                nc.sync.dma_start(out=wo_sb[:, ct, :], in_=woT[ts(ct, P), :])
            ident_sb = consts.tile([P, P], BF)
            nc.scalar.dma_start(out=ident_sb, in_=ident)
            for hh in range(2):
                for ct in range(CT):
                    eng = nc.sync if ct == 0 else nc.gpsimd
                    eng.dma_start(
                        out=x_sb[ct][:, ts(hh, NH // 2)],
                        in_=xb[ts(ct, P), ts(hh, NH // 2)],
                    )
            # constant bias inside exp keeps fp8 attention weights in range
            # (max score/16 ~ 5.5 -> exp up to ~450 overflows e4m3); the e^-2
            # factor cancels exactly in the softmax ratio.
            nexp_sb = consts.tile([P, 1], F32)
            nc.vector.memset(nexp_sb, -2.0)
            # views into the packed constants
            bq_sb = cpack_sb[:, 0:CT]
            bk_sb = cpack_sb[:, CT : 2 * CT]
            bo_sb = cpack_sb[:, 2 * CT : 3 * CT]
            gam_sb = cpack_sb[:, 3 * CT : 4 * CT]
            bet_sb = cpack_sb[:, 4 * CT : 5 * CT]
            mfwd_sb = cpack_sb[:, 10 : 10 + GT]
            mbwd_sb = cpack_sb[0:GT, 26 : 26 + P]

            # ---- persistent activations -----------------------------------
            # q8/k8: channel c lives at (partition c % 128, plane c // 128)
            # so fp8 DoubleRow matmuls contract all 256 channels at once.
            q8_sb = persist.tile([P, CT, NH], F8, tag="q8", name="q8")
            k8_sb = persist.tile([P, CT, N], F8, tag="k8", name="k8")
            h_sb = [persist.tile([P, N], BF, tag=f"h{ct}", name=f"h{ct}") for ct in range(CT)]
            oT_sb = [persist.tile([P, NH], BF, tag=f"oT{ct}", name=f"oT{ct}") for ct in range(CT)]
            vt_tiles = [vt_pool.tile([P, 2, C + 1], F8, tag="vt", name=f"vt{j}") for j in range(PMT)]
            # denominator column: constant 1 plane, written once (gpsimd)
            for j in range(PMT):
                nc.gpsimd.memset(vt_tiles[j][:, :, C : C + 1], 1.0)

            # ---- GroupNorm -------------------------------------------------
            with (
                tc.tile_pool(name="gn_pool", bufs=3) as gn_pool,
                tc.tile_pool(name="gn_psum", bufs=1, space="PSUM") as gn_psum,
            ):
                ab_consts = [None, None]
                st2s = [None, None]
                # asymmetric stats split: the otherwise-idle ACT head sums
                # xh(ct0) and its square (same exp table set, no reload);
                # DVE bn_stats covers xl(ct0) and all of ct1, so the GN
                # chain for ct1 never waits on the ACT accumulators.
                scr = gn_pool.tile([P, NH // 2], F32, tag="scr", name="scr")
                sxp = gn_pool.tile([P, 2], F32, tag="sxp", name="sxp")
                sqp = gn_pool.tile([P, 2], F32, tag="sqp", name="sqp")
                for hh in range(2):
                    nc.scalar.activation(
                        out=scr, in_=xh_sb[0][:, ts(hh, NH // 2)],
                        func=AF.Identity, accum_out=sxp[:, hh : hh + 1],
                    )
                    nc.scalar.activation(
                        out=scr, in_=xh_sb[0][:, ts(hh, NH // 2)],
                        func=AF.Square, accum_out=sqp[:, hh : hh + 1],
                    )
                sx0 = gn_pool.tile([P, 1], F32, tag="sx0", name="sx0")
                sq0 = gn_pool.tile([P, 1], F32, tag="sq0", name="sq0")
                nc.vector.tensor_add(out=sx0, in0=sxp[:, 0:1], in1=sxp[:, 1:2])
                nc.vector.tensor_add(out=sq0, in0=sqp[:, 0:1], in1=sqp[:, 1:2])
                # DVE: xl ct0, then the full ct1 stats
                xr0 = xl_sb[0].rearrange("p (s f) -> p s f", f=512)
                st6_0 = gn_pool.tile([P, NH // 512, 6], F32, tag="st6_0", name="st6_0")
                for s in range(NH // 512):
                    nc.vector.bn_stats(out=st6_0[:, s, :], in_=xr0[:, s, :])
                xr1 = xl_sb[1].rearrange("p (s f) -> p s f", f=512)
                xhr1 = xh_sb[1].rearrange("p (s f) -> p s f", f=512)
                st6_1 = gn_pool.tile([P, N // 512, 6], F32, tag="st6_1", name="st6_1")
                for s in range(NH // 512):
                    nc.vector.bn_stats(out=st6_1[:, s, :], in_=xr1[:, s, :])
                for s in range(NH // 512):
                    nc.vector.bn_stats(
                        out=st6_1[:, NH // 512 + s, :], in_=xhr1[:, s, :]
                    )
                # ct1: plain (mean, E[x^2])
                st2m = gn_pool.tile([P, 4], F32, tag="st2m", name="st2m")
                st2s[0] = st2m[:, 0:2]
                st2s[1] = st2m[:, 2:4]
                mv1 = gn_pool.tile([P, 2], F32, tag="mv1", name="mv1")
                nc.vector.bn_aggr(out=mv1, in_=st6_1)
                nc.vector.tensor_copy(out=st2m[:, 2:3], in_=mv1[:, 0:1])
                msq1 = gn_pool.tile([P, 1], F32, tag="msq1", name="msq1")
                nc.vector.tensor_mul(out=msq1, in0=mv1[:, 0:1], in1=mv1[:, 0:1])
                nc.vector.tensor_add(out=st2m[:, 3:4], in0=mv1[:, 1:2], in1=msq1)
                # ct0: combine the DVE half with the ACT accumulators
                mv0 = gn_pool.tile([P, 2], F32, tag="mv0", name="mv0")
                nc.vector.bn_aggr(out=mv0, in_=st6_0)
                hm = gn_pool.tile([P, 1], F32, tag="hm", name="hm")
                nc.vector.tensor_scalar_mul(out=hm, in0=mv0[:, 0:1], scalar1=0.5)
                nc.vector.scalar_tensor_tensor(
                    out=st2m[:, 0:1], in0=sx0, scalar=0.5 / NH, in1=hm,
                    op0=ALU.mult, op1=ALU.add,
                )
                msq0 = gn_pool.tile([P, 1], F32, tag="msq0", name="msq0")
                nc.vector.tensor_mul(out=msq0, in0=mv0[:, 0:1], in1=mv0[:, 0:1])
                e1 = gn_pool.tile([P, 1], F32, tag="e1", name="e1")
                nc.vector.tensor_add(out=e1, in0=mv0[:, 1:2], in1=msq0)
                nc.vector.tensor_scalar_mul(out=e1, in0=e1, scalar1=0.5)
                nc.vector.scalar_tensor_tensor(
                    out=st2m[:, 1:2], in0=sq0, scalar=0.5 / NH, in1=e1,
                    op0=ALU.mult, op1=ALU.add,
                )
                for ct in (1, 0):
                    st2 = st2s[ct]
                    # per-group (mu, E[x^2]) via 1/8-weighted column sums
                    psum_g = gn_psum.tile([GT, 2], F32, tag="pg")
                    nc.tensor.matmul(psum_g, lhsT=mfwd_sb, rhs=st2, start=True, stop=True)
                    gs = gn_pool.tile([GT, 2], F32, tag="gs")
                    nc.vector.tensor_copy(out=gs[:, 0:1], in_=psum_g[:, 0:1])
                    mq = gn_pool.tile([GT, 1], F32, tag="mq")
                    nc.vector.tensor_mul(out=mq, in0=gs[:, 0:1], in1=gs[:, 0:1])
                    # v = var + eps = (E[x^2] + eps) - mu^2
                    gv = gn_pool.tile([GT, 1], F32, tag="gv")
                    nc.vector.scalar_tensor_tensor(
                        out=gv, in0=psum_g[:, 1:2], scalar=EPS, in1=mq,
                        op0=ALU.add, op1=ALU.subtract,
                    )
                    # 1/sqrt(v): bit-hack seed + one Newton step (all DVE,
                    # keeps the ACT table pinned on exp). Final rel err
                    # ~2e-5; the chain is 6 tiny ops off the exp path.
                    iv = gn_pool.tile([GT, 1], I32, tag="iv")
                    nc.vector.tensor_scalar(
                        out=iv, in0=gv.bitcast(I32), scalar1=1,
                        scalar2=None, op0=ALU.arith_shift_right,
                    )
                    y0i = gn_pool.tile([GT, 1], I32, tag="y0i")
                    nc.vector.tensor_scalar(
                        out=y0i, in0=iv, scalar1=-1, scalar2=RSQRT_MAGIC,
                        op0=ALU.mult, op1=ALU.add,
                    )
                    y0 = y0i.bitcast(F32)
                    t1 = gn_pool.tile([GT, 1], F32, tag="t1")
                    nc.vector.tensor_mul(out=t1, in0=y0, in1=y0)
                    nc.vector.tensor_mul(out=t1, in0=t1, in1=gv)
                    nc.vector.tensor_scalar(
                        out=t1, in0=t1, scalar1=-0.5, scalar2=1.5,
                        op0=ALU.mult, op1=ALU.add,
                    )
                    nc.vector.tensor_mul(out=gs[:, 1:2], in0=y0, in1=t1)
                    # (chain depth note: the Newton step above is 4 serial
                    # ops; the seed alone would pass the tolerance but costs
                    # 3.4% systematic GN scale error - kept for fidelity)
                    # broadcast group stats back to channels
                    psum_bc = gn_psum.tile([P, 2], F32, tag="pbc")
                    nc.tensor.matmul(psum_bc, lhsT=mbwd_sb, rhs=gs, start=True, stop=True)
                    amul = consts.tile([P, 1], F32, tag=f"amul{ct}", name=f"amul{ct}")
                    badd = consts.tile([P, 1], F32, tag=f"badd{ct}", name=f"badd{ct}")
                    nc.vector.tensor_mul(out=amul, in0=psum_bc[:, 1:2], in1=gam_sb[:, ct : ct + 1])
                    nc.vector.tensor_mul(out=badd, in0=psum_bc[:, 0:1], in1=amul)
                    nc.vector.tensor_sub(out=badd, in0=bet_sb[:, ct : ct + 1], in1=badd)
                    ab_consts[ct] = (amul, badd)
                # h = x*A + B (bf16 in/out SBUF -> 4x DVE mode); only the
                # first token piece is written here, the rest interleaves
                # with the projection loop so q0/k0 conversions start sooner
                def h_piece(s4):
                    for ct in range(CT):
                        amul, badd = ab_consts[ct]
                        src_t = xl_sb[ct] if s4 < 2 else xh_sb[ct]
                        sl = ts(s4 % 2, NH // 2)
                        nc.vector.tensor_scalar(
                            out=h_sb[ct][:, ts(s4, N // 4)],
                            in0=src_t[:, sl],
                            scalar1=amul,
                            scalar2=badd,
                            op0=ALU.mult,
                            op1=ALU.add,
                        )

                h_piece(0)

            # ---- projections + attention, software-pipelined --------------
            NCHUNKS = NH // NCH
            with (
                tc.tile_pool(name="p_pool", bufs=64) as p_pool,
                tc.tile_pool(name="s_psum", bufs=2, space="PSUM") as s_psum,
                tc.tile_pool(name="o_psum", bufs=2, space="PSUM") as o_psum,
                tc.tile_pool(name="o_pool", bufs=6) as o_pool,
                tc.tile_pool(name="r_pool", bufs=8) as r_pool,
                tc.tile_pool(name="out_pool", bufs=4) as out_pool,
            ):
                mm_psum = tc.alloc_tile_pool(name="mm_psum", bufs=2, space="PSUM")

                def proj_qk(w_sb, b_sb, dst, ch, on_act=False):
                    for mo in range(CT):
                        psk = mm_psum.tile([P, NCH], F32, tag="psk", name="psk")
                        for ct in range(CT):
                            nc.tensor.matmul(
                                psk,
                                lhsT=w_sb[:, ct, ts(mo, P)],
                                rhs=h_sb[ct][:, ts(ch, NCH)],
                                start=(ct == 0),
                                stop=(ct == CT - 1),
                            )
                        if on_act:
                            # the idle ACT head converts the first key chunk
                            # so the first exp isn't waiting on the DVE queue
                            nc.scalar.activation(
                                out=dst[:, mo, ts(ch, NCH)],
                                in_=psk,
                                func=AF.Identity,
                                bias=b_sb[:, mo : mo + 1],
                                scale=1.0,
                            )
                        else:
                            nc.vector.tensor_scalar_add(
                                out=dst[:, mo, ts(ch, NCH)],
                                in0=psk,
                                scalar1=b_sb[:, mo : mo + 1],
                            )

                def proj_v(j):
                    psv = mm_psum.tile([P, 2, C], F32, tag="psk", name="psv")
                    for parity in range(2):
                        for ct in range(CT):
                            hsl = h_sb[ct][:, ds(j * 2 * P, 2 * P)].rearrange(
                                "p (m two) -> p two m", two=2
                            )
                            nc.tensor.matmul(
                                psv[:, parity, :],
                                lhsT=hsl[:, parity, :],
                                rhs=wv_sb[:, ct, :],
                                start=(ct == 0),
                                stop=(ct == CT - 1),
                            )
                    nc.vector.tensor_copy(out=vt_tiles[j][:, :, 0:C], in_=psv)

                pts_all = [[None] * PMT for _ in range(NCHUNKS)]
                dve_pend = []

                def _ksl(j):
                    return k8_sb[:, :, ds(j * 2 * P, 2 * P)].rearrange(
                        "p r (m two) -> p r two m", two=2
                    )

                def _dve_parity(ch, j, pt, parity, dpool):
                    # fast-exp path: its S matmul gets a private PSUM bank so
                    # the ACT exp stream's two-slot rotation never skips
                    psd = dpool.tile([P, NCH], F32, tag="dves", name=f"psd{ch}_{j}_{parity}")
                    nc.tensor.matmul(
                        psd,
                        lhsT=_ksl(j)[:, :, parity, :],
                        rhs=q8_sb[:, :, ts(ch, NCH)],
                        start=True,
                        stop=True,
                        perf_mode=mybir.MatmulPerfMode.DoubleRow,
                    )
                    nc.vector.tensor_scalar(
                        out=pt.bitcast(U8)[:, parity, :],
                        in0=psd,
                        scalar1=FE_MUL,
                        scalar2=FE_OFF,
                        op0=ALU.mult,
                        op1=ALU.add,
                    )

                def dve_drain():
                    while dve_pend:
                        ch, j, pt, dpool = dve_pend.pop(0)
                        _dve_parity(ch, j, pt, 1, dpool)

                def s_exp(ch, j, dpool=None):
                    on_dve = dpool is not None and j in DVE_J[ch]
                    pt = p_pool.tile(
                        [P, 2, NCH], F8E5 if on_dve else F8,
                        tag="pt", name=f"pt{ch}_{j}",
                    )
                    pts_all[ch][j] = pt
                    if on_dve:
                        # parity 0 now; parity 1 next gap (the single bank
                        # round-trips through the DVE in between)
                        _dve_parity(ch, j, pt, 0, dpool)
                        dve_pend.append((ch, j, pt, dpool))
                        return
                    pss = s_psum.tile([P, 2, NCH], F32, tag="pss")
                    for parity in range(2):
                        nc.tensor.matmul(
                            pss[:, parity, :],
                            lhsT=_ksl(j)[:, :, parity, :],
                            rhs=q8_sb[:, :, ts(ch, NCH)],
                            start=True,
                            stop=True,
                            perf_mode=mybir.MatmulPerfMode.DoubleRow,
                        )
                    nc.scalar.activation(
                        out=pt, in_=pss, func=AF.Exp, scale=SCALE, bias=nexp_sb
                    )

                # PV bookkeeping: psos[(ch, half)] -> [pso_g0, pso_g1]
                psos = {}

                def pv2(ch, half, jj):
                    key = (ch, half)
                    if key not in psos:
                        psos[key] = [
                            o_psum.tile([P, C + 1], F32, tag="pso", name=f"pso{ch}_{half}_{g}")
                            for g in range(2)
                        ]
                    for g in range(2):
                        nt = 2 * half + g
                        nc.tensor.matmul(
                            psos[key][g],
                            lhsT=pts_all[ch][jj][:, :, ts(nt, P)],
                            rhs=vt_tiles[jj],
                            start=(jj == 0),
                            stop=(jj == PMT - 1),
                            perf_mode=mybir.MatmulPerfMode.DoubleRow,
                        )

                osb_pend = {}

                def finish_dve(ch, half, on_act=False):
                    # divide the accumulated o^T by the softmax denominator
                    # (on_act: the post-exp tail runs the divides on the
                    # now-idle ACT engine, only the reciprocals stay DVE)
                    pp = psos.pop((ch, half))
                    osbs = []
                    for g in range(2):
                        nt = 2 * half + g
                        rec = r_pool.tile([P, 1], F32, tag="rec", name=f"rec{ch}_{nt}")
                        nc.vector.reciprocal(out=rec, in_=pp[g][:, C : C + 1])
                        osb = o_pool.tile([P, C], BF, tag="osb", name=f"osb{ch}_{nt}")
                        if on_act:
                            nc.scalar.activation(
                                out=osb, in_=pp[g][:, 0:C], func=AF.Identity, scale=rec
                            )
                        else:
                            nc.vector.tensor_scalar_mul(out=osb, in0=pp[g][:, 0:C], scalar1=rec)
                        osbs.append(osb)
                    osb_pend[(ch, half)] = osbs

                def finish_pe(ch, half, pool, pst_tag="psf", on_act=False):
                    # transpose o^T -> o (emitted a gap later than the DVE
                    # part so the PE never blocks waiting on osb)
                    osbs = osb_pend.pop((ch, half))
                    for cc in range(CT):
                        pst = pool.tile([P, 2, P], BF, tag=pst_tag, name=f"pst{ch}{half}{cc}")
                        for g in range(2):
                            nc.tensor.transpose(pst[:, g, :], osbs[g][:, ts(cc, P)], ident_sb)
                        if on_act:
                            nc.scalar.activation(
                                out=oT_sb[cc][:, ds(ch * NCH + half * 2 * P, 2 * P)],
                                in_=pst,
                                func=AF.Identity,
                            )
                        else:
                            nc.vector.tensor_copy(
                                out=oT_sb[cc][:, ds(ch * NCH + half * 2 * P, 2 * P)],
                                in_=pst,
                            )

                def proj_mo(ch, mo, pool, tag="psf", pieces=1):
                    psf = pool.tile([P, NCH], F32, tag=tag, name=f"psj{ch}{mo}")
                    for ct in range(CT):
                        nc.tensor.matmul(
                            psf,
                            lhsT=wo_sb[:, ct, ts(mo, P)],
                            rhs=oT_sb[ct][:, ts(ch, NCH)],
                            start=(ct == 0),
                            stop=(ct == CT - 1),
                        )
                    pw = NCH // pieces
                    for pp in range(pieces):
                        fs = out_pool.tile([P, pw], F32, tag="fs", name=f"fs{ch}{mo}{pp}")
                        nc.vector.scalar_tensor_tensor(
                            out=fs,
                            in0=psf[:, ts(pp, pw)],
                            scalar=bo_sb[:, mo : mo + 1],
                            in1=x_sb[mo][:, ds(ch * NCH + pp * pw, pw)],
                            op0=ALU.add,
                            op1=ALU.add,
                        )
                        eng = nc.sync if (mo + pp) % 2 == 0 else nc.gpsimd
                        eng.dma_start(
                            out=out[ts(mo, P), ds(ch * NCH + pp * pw, pw)], in_=fs
                        )

                def proj_half(ch, mo, half, pool, tag="psf"):
                    # 256-wide output projection keyed to one finish half so
                    # the tail's DMAs launch as soon as that half's o lands
                    base = ch * NCH + half * 2 * P
                    psf = pool.tile([P, 2 * P], F32, tag=tag, name=f"psh{ch}{mo}{half}")
                    for ct in range(CT):
                        nc.tensor.matmul(
                            psf,
                            lhsT=wo_sb[:, ct, ts(mo, P)],
                            rhs=oT_sb[ct][:, ds(base, 2 * P)],
                            start=(ct == 0),
                            stop=(ct == CT - 1),
                        )
                    fs = out_pool.tile([P, 2 * P], F32, tag="fs", name=f"fsh{ch}{mo}{half}")
                    nc.vector.scalar_tensor_tensor(
                        out=fs,
                        in0=psf,
                        scalar=bo_sb[:, mo : mo + 1],
                        in1=x_sb[mo][:, ds(base, 2 * P)],
                        op0=ALU.add,
                        op1=ALU.add,
                    )
                    eng = nc.sync if (mo + half) % 2 == 0 else nc.gpsimd
                    eng.dma_start(out=out[ts(mo, P), ds(base, 2 * P)], in_=fs)

                # -- window 0: projections + chunk-0 exps interleaved.
                # DVE conversion order: q0, k0(ACT), k1..k7 (h pieces
                # slotted in), then q1..q3 and v interleaved so the key
                # tiles keep pace with the exps and vT lands just ahead of
                # chunk 0's PV in window 1.
                proj_qk(wq_sb, bq_sb, q8_sb, 0)
                for pc in range(8):
                    if pc >= 4:
                        proj_v(pc - 4)
                    proj_qk(wk_sb, bk_sb, k8_sb, pc, on_act=(pc == 0))
                    s_exp(0, 2 * pc)
                    s_exp(0, 2 * pc + 1)
                    if pc in (1, 3, 5):
                        h_piece(pc // 2 + 1)
                proj_qk(wq_sb, bq_sb, q8_sb, 1)
                proj_v(4)
                proj_v(5)
                proj_qk(wq_sb, bq_sb, q8_sb, 2)
                proj_v(6)
                proj_v(7)
                proj_qk(wq_sb, bq_sb, q8_sb, 3)
                for j in range(8, PMT):
                    proj_v(j)
                mm_psum.release()

                # -- windows 1..3: exps + displaced PV pipeline -------------
                # Gap map for window ch (prev = ch-1, pp = ch-2):
                #   g0:    pv(prev,h0) 0-1   + finish_dve(pp, h1)
                #   g1:    pv(prev,h0) 2-3   + finish_pe(pp, h1)
                #   g2:    pv(prev,h0) 4-5   + proj_mo(pp, 0)
                #   g3:    pv(prev,h0) 6-7   + proj_mo(pp, 1)
                #   g4-7:  pv(prev,h0) 8-15
                #   g8:    finish_dve(prev, h0)
                #   g9:    finish_pe(prev, h0) + pv(prev,h1) 0-1
                #   g10-15: pv(prev,h1) 2-15 (2-3 per gap); in the last
                #          window also chunk-3 h0 accumulations (exp-gated)
                # (tf_psum and d_psum reuse the two banks mm_psum released)
                with (
                    tc.tile_pool(name="tf_psum", bufs=1, space="PSUM") as tf_psum,
                    tc.tile_pool(name="d_psum", bufs=1, space="PSUM") as d_psum,
                ):
                    lc = NCHUNKS - 1
                    h0_pend = list(range(PMT))  # last chunk's half-0 PV queue
                    # window 1 runs while the DVE is still draining the q/k/v
                    # conversion backlog, so chunk 0 gets NO finish work there:
                    # its half-1 PV accumulates in the (idle until window 2)
                    # tf/d banks and all four finishes open window 2 instead.
                    for j in range(PMT):
                        s_exp(1, j, d_psum)
                        if j < 8:
                            pv2(0, 0, 2 * j)
                            pv2(0, 0, 2 * j + 1)
                        else:
                            if j == 8:
                                psos[(0, 1)] = [
                                    tf_psum.tile([P, NCH], F32, tag="psf", name="ps01a")[:, 0 : C + 1],
                                    d_psum.tile([P, NCH], F32, tag="dves", name="ps01b")[:, 0 : C + 1],
                                ]
                            pv2(0, 1, 2 * (j - 8))
                            pv2(0, 1, 2 * (j - 8) + 1)
                    for ch in (2, 3):
                        prev = ch - 1
                        last = ch == lc
                        for j in range(PMT):
                            dve_drain()
                            s_exp(ch, j, d_psum)
                            if ch == 2 and j < 4:
                                # chunk 0's deferred finishes
                                if j == 0:
                                    finish_dve(0, 0)
                                elif j == 1:
                                    finish_dve(0, 1)
                                elif j == 2:
                                    finish_pe(0, 0, tf_psum)
                                else:
                                    finish_pe(0, 1, tf_psum)
                            if ch == 3 and j < 2:
                                proj_mo(1, j, tf_psum)
                            if j < 4:
                                for jj in range(4 * j, 4 * j + 4):
                                    pv2(prev, 0, jj)
                            elif j == 4:
                                finish_dve(prev, 0)
                            elif j == 5:
                                finish_pe(prev, 0, tf_psum)
                                pv2(prev, 1, 0)
                                pv2(prev, 1, 1)
                            elif j <= 12:
                                if ch == 2 and j == 6:
                                    proj_mo(0, 0, tf_psum)
                                elif ch == 2 and j == 7:
                                    proj_mo(0, 1, tf_psum)
                                pv2(prev, 1, 2 * j - 10)
                                pv2(prev, 1, 2 * j - 9)
                            elif j == 13:
                                finish_dve(prev, 1)
                            elif j == 14:
                                finish_pe(prev, 1, tf_psum)
                            if last and j >= 6:
                                # chunk-3 half-0: accumulate exp-ready tiles
                                # (fast-exp tiles are complete a gap later)
                                take = [
                                    jj for jj in h0_pend
                                    if jj < j and (jj not in DVE_J[lc] or jj < j - 1)
                                ][:3]
                                for jj in take:
                                    h0_pend.remove(jj)
                                    pv2(lc, 0, jj)
                        dve_drain()

                    # tail: the exp rotation is over, so chunk-3's half-1 PV
                    # accumulates in the two dead s_psum slots (separate
                    # tiles - slice-sharing one tile serializes the
                    # accumulation on the PSUM read-modify-write latency).
                    # Only its last tiles gate on the final exps; nothing
                    # waits on chunk-2's banks.
                    proj_mo(lc - 1, 0, tf_psum)
                    proj_mo(lc - 1, 1, d_psum, tag="dves")
                    psos[(lc, 1)] = [
                        s_psum.tile([P, 2, NCH], F32, tag="pss", name=f"pso31_{g}")[
                            :, 0, 0 : C + 1
                        ]
                        for g in range(2)
                    ]
                    for jj in range(PMT):
                        pv2(lc, 1, jj)
                    for jj in h0_pend:
                        pv2(lc, 0, jj)
                    finish_dve(lc, 0)
                    finish_dve(lc, 1, on_act=True)
                    finish_pe(lc, 0, tf_psum)
                    proj_half(lc, 0, 0, tf_psum)
                    proj_half(lc, 1, 0, d_psum, tag="dves")
                    finish_pe(lc, 1, o_psum, pst_tag="pso", on_act=True)
                    proj_half(lc, 0, 1, tf_psum)
                    proj_half(lc, 1, 1, d_psum, tag="dves")

    nc.compile()
    return nc


def get_program():
    if "nc" not in _CACHE:
        _CACHE["nc"] = _build_program()
    return _CACHE["nc"]


def _cpack(bq, bk, bo2, gam, bet):
    cp = np.zeros((P, 10 + 16 + P), np.float32)
    for j, v in enumerate([bq, bk, bo2, gam, bet]):
        cp[:, 2 * j : 2 * j + 2] = v.reshape(CT, P).T
    mfwd = (
        np.arange(P)[:, None] // GSIZE == np.arange(GROUPS // CT)[None, :]
    ).astype(np.float32) / GSIZE
    mbwd = (
        np.arange(GROUPS // CT)[:, None] == np.arange(P)[None, :] // GSIZE
    ).astype(np.float32)
    cp[:, 10:26] = mfwd
    cp[: GROUPS // CT, 26 : 26 + P] = mbwd
    return cp


def _make_in_maps(x, gn_gamma, gn_beta, wq, bq, wk, bk, wv, bv, wo, bo):
    f = lambda a: np.ascontiguousarray(np.asarray(a, dtype=np.float32))
    x = f(x).reshape(B, C, N)
    # v's bias is constant across tokens, so it rides through attention
    # unchanged: softmax(S) @ (v + bv) = softmax(S) @ v + bv. Fold it into
    # the output projection bias: bo' = wo @ bv + bo (exact fp32 host math).
    bo2 = f(wo) @ f(bv) + f(bo)
    shared = {
        "wqT": f(wq).T.astype(ml_dtypes.bfloat16),
        "wkT": f(wk).T.astype(ml_dtypes.bfloat16),
        "wvT": np.ascontiguousarray(f(wv).T).astype(ml_dtypes.bfloat16),
        "woT": f(wo).T.astype(ml_dtypes.bfloat16),
        "cpack": _cpack(f(bq), f(bk), bo2, f(gn_gamma), f(gn_beta)),
        "ident": np.eye(P).astype(ml_dtypes.bfloat16),
    }
    in_maps = []
    for core in range(8):
        b, half = core // 2, core % 2
        xbv = x[b]
        if half == 1:
            xbv = np.concatenate([xbv[:, NH:], xbv[:, :NH]], axis=1)
        in_maps.append(
            {
                "xb": np.ascontiguousarray(xbv[:, :NH]),
                "xlb": xbv[:, :NH].astype(ml_dtypes.bfloat16),
                "xhb": xbv[:, NH:].astype(ml_dtypes.bfloat16),
                **shared,
            }
        )
    return in_maps


def kernel(**inputs):
    nc = get_program()
    in_maps = _make_in_maps(**inputs)
    res = run_bass_kernel_spmd(nc, in_maps, list(range(8)))
    out = np.empty((B, C, N), dtype=np.float32)
    for core in range(8):
        b, half = core // 2, core % 2
        out[b, :, half * NH : (half + 1) * NH] = res.results[core]["out"]
    return out.reshape(B, C, W, W)
